# revision 16
# baseline (speedup 1.0000x reference)
"""Trainium2 Bass kernel for HNet dechunk (EMA over boundary-selected tokens), v5.

Per-position recurrence (equivalent to the reference, verified in v1):
    q_t    = mask_t * clip(p_t, EPS, 1-EPS)
    pbi_t  = cumsum(mask)_t - 1
    out[t] = (1 - q_t) * out[t-1] + q_t * hidden[pbi_t]

Sharding: 8 cores = 4 batch rows x 2 halves of D.

v5 = v4 (127-position blocks, carry on contraction row 127, group-built
transition matrices, big dma_gathers, f32r matmuls, bf16 output) with the
two passes software-pipelined at half granularity:
  * W2 (65x65 block-level transition products) depends only on p/m, so it
    is built in prep. The carry-inclusive end states H are then computed
    per 32-block half by 1-2 matmuls against the accumulated local states
    (contraction split at partition 32, keeping all operand base
    partitions in {0, 32}).
  * Schedule: prep -> pass A half 0 -> H half 0 + plant -> {pass A half 1
    interleaved with pass B half 0} -> H half 1 + plant -> pass B half 1.
    Pass B's evicts and output DMAs fill the engine/DMA idle time that a
    full A->mid->B barrier would leave.
  * Fixed engine assignment: DVE = qd chunks + scans + evict share;
    Pool = d0t chunks + gather descgen + memsets; Act = wl-column copies +
    PSUM->SBUF copies + evict share; PE = transposes + matmuls.
"""

from contextlib import ExitStack

import ml_dtypes
import numpy as np

import concourse.bass as bass
import concourse.tile as tile
from concourse import bacc, mybir
from concourse.bass_utils import run_bass_kernel_spmd
from concourse.masks import make_identity, make_lower_triangular

EPS = 1e-4
P = 128
PB = 127          # positions per block
FP = mybir.dt.float32
FR = mybir.dt.float32r
BF = mybir.dt.bfloat16
B, L, D = 4, 8192, 1024
NCORES = 8
DC = 512          # channels per core (D / 2)
NB = 65           # ceil(L / PB); 65*127 = 8255
NFULL = 64        # blocks fully inside L
LTAIL = L - NFULL * PB  # 64 tokens in the last block
GG = 13           # blocks per gather DMA (5 gathers)
NCH = 4           # blocks per group op
NG = (NB + NCH - 1) // NCH  # 17 groups; groups 0..7 = half 0, 8..16 = half 1

_cache: dict = {}

Alu = mybir.AluOpType
Act = mybir.ActivationFunctionType


def _emit(tc, ctx, x_ap, p_ap, m_ap, idx_ap, out_ap):
    nc = tc.nc

    const = ctx.enter_context(tc.tile_pool(name="const", bufs=1))
    prep = ctx.enter_context(tc.tile_pool(name="prep", bufs=1))
    big = ctx.enter_context(tc.tile_pool(name="big", bufs=1))
    small = ctx.enter_context(tc.tile_pool(name="small", bufs=2))
    chunk = ctx.enter_context(tc.tile_pool(name="chunk", bufs=2))
    stage = ctx.enter_context(tc.tile_pool(name="stage", bufs=4))
    psum_mm = ctx.enter_context(tc.tile_pool(name="pmm", bufs=2, space="PSUM"))
    actx = ctx.enter_context(ExitStack())
    psum_tr = actx.enter_context(tc.tile_pool(name="ptr", bufs=2, space="PSUM"))
    psum_hl0 = actx.enter_context(tc.tile_pool(name="phl0", bufs=1, space="PSUM"))
    psum_hl1 = actx.enter_context(tc.tile_pool(name="phl1", bufs=1, space="PSUM"))

    # ---- constants ----
    ident = const.tile([P, P], FP)
    make_identity(nc, ident[:])
    # lt2[j, k] = 1 iff k < j; col 127 = [j >= 1] (carry-row transition after
    # transpose; zero at block starts so group scans reset there).
    lt2 = const.tile([P, P], FP)
    make_lower_triangular(nc, lt2[:], val=1.0, diag=False)
    nc.vector.tensor_copy(lt2[:, P - 1:P], lt2[:, 0:1])
    one1 = const.tile([1, 1], FP)
    nc.gpsimd.memset(one1[:], 1.0)
    # ident2 = ident plus a 1 at [127, 0] (carry-row seed selector)
    ident2 = const.tile([P, P], FP)
    nc.vector.tensor_copy(ident2[:], ident[:])
    nc.sync.dma_start(ident2[P - 1:P, 0:1], one1[:])
    ones = const.tile([NB, P], FP)
    nc.gpsimd.memset(ones[:], 1.0)

    # ---- gather indices + big gathers (start ASAP) ----
    NI = NB * P
    idx16 = prep.tile([P, NI // 16], mybir.dt.int16)
    nc.sync.dma_start(idx16[:], idx_ap)

    # ---- gathers ----
    # two tiles (one per 32-block half) so the carry-plant DMAs only form
    # WAW dependencies (tile-granular for DMA-DMA) with their own half
    xg0 = big.tile([P, 32, DC], BF)
    xg1 = big.tile([P, NB - 32, DC], BF)

    def xg_slot(c):
        return xg0[:, c, :] if c < 32 else xg1[:, c - 32, :]

    def emit_gather(lo, hi, q):
        dst = xg0[:, lo:hi, :] if hi <= 32 else xg1[:, lo - 32:hi - 32, :]
        nc.gpsimd.dma_gather(
            dst,
            x_ap,
            idx16[:, lo * P // 16:hi * P // 16],
            num_idxs=(hi - lo) * P,
            num_idxs_reg=(hi - lo) * P,
            elem_size=DC,
            queue_num=q,
        )

    # half-0 gathers first (descgen waits only on idx16); queue chaining
    # doubles as arbitration control: the DMA device grants in
    # FIFO-of-attempt order and a queue only attempts its head once the
    # previous entry completes, so stacking deep-half gathers behind each
    # other lets the tiny H-plant DMAs slip in between.
    # the SWDGE ring holds 1024 descriptors; gathers above that hang the
    # ucode on hardware, so every gather is capped at 8 blocks (1024 rows)
    emit_gather(0, 8, 0)
    emit_gather(8, 16, 1)
    emit_gather(16, 24, 0)
    emit_gather(24, 32, 1)

    # ---- per-position scalars, (block, pos-in-block) layout ----
    m_u8 = prep.tile([NB, P], mybir.dt.uint8)
    nc.gpsimd.memset(m_u8[:], 0)
    nc.sync.dma_start(m_u8[:NFULL, :PB],
                      m_ap[:NFULL * PB].rearrange("(a b) -> a b", b=PB))
    nc.sync.dma_start(m_u8[NFULL:NB, :LTAIL], m_ap[NFULL * PB:L])
    mt = prep.tile([NB, P], FP)
    nc.vector.tensor_copy(mt[:], m_u8[:])
    pt = prep.tile([NB, P], FP)
    nc.gpsimd.memset(pt[:], 0.0)
    nc.sync.dma_start(pt[:NFULL, :PB],
                      p_ap[:NFULL * PB].rearrange("(a b) -> a b", b=PB))
    nc.sync.dma_start(pt[NFULL:NB, :LTAIL], p_ap[NFULL * PB:L])
    pc = prep.tile([NB, P], FP)
    nc.vector.tensor_scalar(pc[:], pt[:], 1.0 - EPS, EPS,
                            op0=Alu.min, op1=Alu.max)
    qt = prep.tile([NB, P], FP)
    nc.vector.tensor_tensor(out=qt[:], in0=mt[:], in1=pc[:], op=Alu.mult)
    at = prep.tile([NB, P], FP)
    nc.vector.tensor_scalar(at[:], qt[:], -1.0, 1.0,
                            op0=Alu.mult, op1=Alu.add)
    # per-block total decay: cumprod, last column
    acum = prep.tile([NB, P], FP)
    nc.vector.tensor_tensor_scan(acum[:], at[:], ones[:], 1.0,
                                 op0=Alu.mult, op1=Alu.mult)
    ablk = acum[:, P - 1:P]

    # transposes to (pos-in-block, block) layout
    qT_ps = psum_tr.tile([P, NB], FP, space="PSUM", tag="tr")
    nc.tensor.transpose(qT_ps[:], qt[:], ident[:NB, :NB])
    qT = prep.tile([P, NB], FP)
    nc.scalar.activation(qT[:], qT_ps[:], Act.Copy)
    aT_ps = psum_tr.tile([P, NB], FP, space="PSUM", tag="tr")
    nc.tensor.transpose(aT_ps[:], at[:], ident[:NB, :NB])
    aT = prep.tile([P, NB], FP)
    nc.scalar.activation(aT[:], aT_ps[:], Act.Copy)

    # seeds = qT with row 127 replaced by each block's first a value
    # seeds row 127: block c's carry-row seed = a at its first position.
    # Slot 0 keeps qT[127, 0] = 0 (pad q), so block 0's carry row is
    # identically zero and the garbage on its rhs partition 127 is ignored
    # (H[-1] = 0 without any extra zeroing DMA).
    seeds = prep.tile([P, NB], FP)
    nc.vector.tensor_copy(seeds[:], qT[:])
    nc.sync.dma_start(seeds[P - 1:P, 1:NB], aT[0:1, 1:NB])

    # W2[c', c] = prod_{j=c'+1..c} Ablk_j (c >= c'); depends only on p/m.
    # Only rows 0..31 are ever contracted (the half-1 x half-1 term uses a
    # separate base-0 scan w2b because matmul operands cannot sit at
    # partition offset 32).
    d02t = small.tile([NB, 32], FP, tag="d02")
    nc.gpsimd.tensor_tensor(out=d02t[:], in0=ablk.to_broadcast([NB, 32]),
                            in1=lt2[:NB, :32], op=Alu.mult)
    d02_ps = psum_tr.tile([32, NB], FP, space="PSUM", tag="tr")
    nc.tensor.transpose(d02_ps[:], d02t[:], ident[:NB, :NB])
    w2 = prep.tile([32, NB], BF)
    nc.vector.tensor_tensor_scan(w2[:], d02_ps[:], ident[:32, :NB], 0.0,
                                 op0=Alu.mult, op1=Alu.add)
    # ablk rows 32..63 relocated to partitions 0..31 via two transposes
    ablkr_ps = psum_tr.tile([1, NB], FP, space="PSUM", tag="tr")
    nc.tensor.transpose(ablkr_ps[:], ablk, ident[:NB, :NB])
    ablkr = prep.tile([1, NB], FP)
    nc.vector.tensor_copy(ablkr[:], ablkr_ps[:])
    ablkb_ps = psum_tr.tile([32, 1], FP, space="PSUM", tag="tr")
    nc.tensor.transpose(ablkb_ps[:], ablkr[:, 32:64], ident[:1, :1])
    ablkb = prep.tile([32, 1], FP)
    nc.vector.tensor_copy(ablkb[:], ablkb_ps[:])
    d02tb = small.tile([32, 32], FP, tag="d02b")
    nc.gpsimd.tensor_tensor(out=d02tb[:], in0=ablkb[:].to_broadcast([32, 32]),
                            in1=lt2[:32, :32], op=Alu.mult)
    d02b_ps = psum_tr.tile([32, 32], FP, space="PSUM", tag="tr")
    nc.tensor.transpose(d02b_ps[:], d02tb[:], ident[:32, :32])
    w2b = prep.tile([32, 32], BF)
    nc.vector.tensor_tensor_scan(w2b[:], d02b_ps[:], ident[:32, :32], 0.0,
                                 op0=Alu.mult, op1=Alu.add)


    # half-0 gathers now; half-1 gathers are emitted progressively later so
    # their descgen (readiness) trails the H-plant DMAs in the DMA-engine
    # arbitration queue (first-ready-first-served).

    # ---- pass A/B machinery ----
    wqall = big.tile([P, NB * P], BF)
    wlar = big.tile([P, NB * 32], BF)  # pre-zeroed column-isolated lhsTs
    zcol = const.tile([P, 1], FP)
    nc.gpsimd.memset(zcol[:], 0.0)
    hl_ps = [psum_hl0.tile([32, DC], FP, space="PSUM", tag="hl0", name="hl0"),
             psum_hl1.tile([32, DC], FP, space="PSUM", tag="hl1", name="hl1")]
    hsb = [prep.tile([32, DC], BF, name="hsb0"),
           prep.tile([32, DC], BF, name="hsb1")]
    hs = [prep.tile([32, DC], BF, name="hs0"),
          prep.tile([32, DC], BF, name="hs1")]

    nc.gpsimd.tensor_scalar(wlar[:], zcol[:].to_broadcast([P, NB * 32]),
                            0.0, None, op0=Alu.mult)

    def emit_a_group(g):
        lo = g * NCH
        hi = min(lo + NCH, NB)
        n = hi - lo
        # qd[k, (c,j)] = ident2[k, j] * seeds[k, c]
        qdc = chunk.tile([P, NCH * P], FP, tag="qd")
        nc.vector.tensor_tensor(
            out=qdc[:, :n * P].rearrange("p (c j) -> p c j", j=P),
            in0=ident2[:].unsqueeze(1).broadcast_to([P, n, P]),
            in1=seeds[:, lo:hi].unsqueeze(2).broadcast_to([P, n, P]),
            op=Alu.mult)
        # d0t[j, (c,k)] = a_{c,j} * lt2[j, k]
        d0c = chunk.tile([P, NCH * P], FP, tag="d0")
        nc.gpsimd.tensor_tensor(
            out=d0c[:, :n * P].rearrange("p (c j) -> p c j", j=P),
            in0=aT[:, lo:hi].unsqueeze(2).broadcast_to([P, n, P]),
            in1=lt2[:].unsqueeze(1).broadcast_to([P, n, P]),
            op=Alu.mult)
        d0_ps = psum_tr.tile([P, NCH * P], FP, space="PSUM", tag="tr")
        for j in range(n):
            nc.tensor.transpose(d0_ps[:, j * P:(j + 1) * P],
                                d0c[:, j * P:(j + 1) * P], ident[:])
        # one scan per group; block-start columns of d0_ps are all zero,
        # so the recurrence resets at every block boundary.
        nc.vector.tensor_tensor_scan(wqall[:, lo * P:hi * P],
                                     d0_ps[:, :n * P], qdc[:, :n * P], 0.0,
                                     op0=Alu.mult, op1=Alu.add)

    def emit_hloc_group(g):
        lo = g * NCH
        hi = min(lo + NCH, NFULL)  # block 64's local state is never used
        for c in range(lo, hi):
            col = c * 32 + (c % 32)
            nc.scalar.activation(wlar[:PB, col:col + 1],
                                 wqall[:PB, c * P + PB - 1:c * P + PB],
                                 Act.Copy)
            nc.tensor.matmul(hl_ps[c // 32][:],
                             wlar[:, c * 32:c * 32 + 32],
                             xg_slot(c),
                             start=(c % 32 == 0), stop=(c % 32 == 31),
                             skip_group_check=True)

    def emit_mid(h):
        # H[32h .. 32h+31] from accumulated local states; plant into
        # partition 127 of the next half's rhs slots.
        nc.scalar.activation(hsb[h][:], hl_ps[h][:], Act.Copy)
        # H tiles ride the freed hl pools (ring reuse after the hsb copy)
        pool = psum_hl0 if h == 0 else psum_hl1
        H_ps = pool.tile([32, DC], FP, space="PSUM",
                         tag="hl0" if h == 0 else "hl1", name="H_ps")
        if h == 0:
            nc.tensor.matmul(H_ps[:], w2[0:32, 0:32], hsb[0][:],
                             start=True, stop=True)
        else:
            nc.tensor.matmul(H_ps[:], w2[0:32, 32:64], hsb[0][:],
                             start=True, stop=False)
            nc.tensor.matmul(H_ps[:], w2b[:], hsb[1][:],
                             start=False, stop=True)
        nc.scalar.activation(hs[h][:], H_ps[:], Act.Copy)
        # Act HWDGE queue: only tiny DMAs live here, so these attempt the
        # DMA device at readiness instead of queueing behind gathers/outputs
        if h == 0:
            nc.scalar.dma_start(xg0[P - 1:P, 1:32, :], hs[0][0:31, :])
            nc.scalar.dma_start(xg1[P - 1:P, 0:1, :], hs[0][31:32, :])
        else:
            nc.scalar.dma_start(xg1[P - 1:P, 1:NB - 32, :],
                                hs[1][0:NB - 33, :])

    nev = [0]
    mm_pools = [psum_mm]

    def emit_b_group(g):
        lo = g * NCH
        hi = min(lo + NCH, NB)
        for half in range(2):
            j0 = lo + half * 2
            j1 = min(j0 + 2, hi)
            if j0 >= j1:
                continue
            n = j1 - j0
            pool = mm_pools[nev[0] % len(mm_pools)]
            mm2 = pool.tile([P, 2 * DC], FP, space="PSUM", tag="mm",
                            name="mm2")
            for j in range(n):
                c = j0 + j
                nc.tensor.matmul(mm2[:, j * DC:(j + 1) * DC],
                                 wqall[:, c * P:(c + 1) * P],
                                 xg_slot(c),
                                 start=True, stop=True, skip_group_check=True)
            if j1 <= NFULL:
                st = stage.tile([P, 2 * DC], BF, tag="st", name="st")
                if nev[0] % 2 == 0:
                    nc.scalar.activation(st[:, :n * DC], mm2[:, :n * DC],
                                         Act.Copy)
                else:
                    nc.vector.tensor_copy(st[:, :n * DC], mm2[:, :n * DC])
                ov = out_ap[j0 * PB:j1 * PB, :].rearrange(
                    "(c k) d -> k c d", k=PB)
                # SWDGE (gpsimd) for bulk stores: its descgen aggregates the
                # per-partition 1KB descriptors into ~59KB packets that drain
                # at ~240GB/s, while the HWDGE dynamic queues drain them one
                # packet per descriptor at ~41ns/KB (24GB/s) — measured.
                nc.gpsimd.dma_start(ov, st[:PB, :n * DC].rearrange(
                    "k (c d) -> k c d", d=DC))
                nev[0] += 1
            else:
                stt = stage.tile([LTAIL, DC], BF, tag="tail")
                nc.scalar.activation(stt[:], mm2[:LTAIL, :DC], Act.Copy)
                nc.gpsimd.dma_start(out_ap[NFULL * PB:L, :], stt[:])

    # ---- schedule ----
    HG = 8  # groups per half
    # pass A half 0
    for g in range(HG):
        emit_a_group(g)
        if g >= 1:
            emit_hloc_group(g - 1)
    emit_hloc_group(HG - 1)
    emit_gather(32, 40, 0)
    emit_mid(0)
    # pass A half 1 interleaved with pass B half 0; remaining gather chunks
    # are emitted inside the loop so their readiness trails the plant DMA
    for i in range(NG - HG):
        emit_a_group(HG + i)
        if i == 0:
            emit_gather(40, 48, 1)
        elif i == 1:
            emit_gather(48, 56, 0)
        elif i == 2:
            emit_gather(56, 64, 1)
        elif i == 3:
            emit_gather(64, NB, 0)
        if i >= 1:
            emit_hloc_group(HG + i - 1)
        if i < HG:
            emit_b_group(i)
    emit_hloc_group(NG - 1)
    emit_mid(1)
    # pass A psum pools are dead now; free their banks for extra pass-B
    # double buffering
    actx.close()
    psum_mm2 = ctx.enter_context(tc.tile_pool(name="pmm2", bufs=2,
                                              space="PSUM"))
    mm_pools.append(psum_mm2)
    # pass B half 1
    for g in range(HG, NG):
        emit_b_group(g)


def _build(reps=1):
    nc = bacc.Bacc(num_swdge_queues=2)
    x = nc.dram_tensor("x", (L, DC), BF, kind="ExternalInput")
    p = nc.dram_tensor("p", (L,), FP, kind="ExternalInput")
    m = nc.dram_tensor("m", (L,), mybir.dt.uint8, kind="ExternalInput")
    idx = nc.dram_tensor("idx", (P, NB * P // 16), mybir.dt.int16,
                         kind="ExternalInput")
    out = nc.dram_tensor("out", (L, DC), BF, kind="ExternalOutput")
    with tile.TileContext(nc) as tc:
        for _ in range(reps):
            with ExitStack() as ctx:
                _emit(tc, ctx, x[:], p[:], m[:], idx[:], out[:])
    nc.compile()
    return nc


def _make_idx(boundary_mask_row):
    pbi = np.clip(np.cumsum(boundary_mask_row.astype(np.int64)) - 1,
                  0, None).astype(np.int16)
    i = np.arange(NB * P)
    c, j = i // P, i % P
    t = c * PB + j
    valid = (j < PB) & (t < L)
    idx = np.where(valid, pbi[np.minimum(t, L - 1)], 0).astype(np.int16)
    wrapped = idx.reshape(NB * P // 16, 16).T  # (16, NI//16)
    return np.ascontiguousarray(np.tile(wrapped, (8, 1)))  # replicated x8


def _in_maps(hidden_states, boundary_prob, boundary_mask):
    in_maps = []
    for c in range(NCORES):
        b, dh = c // 2, c % 2
        if dh == 0:
            idx16 = _make_idx(np.asarray(boundary_mask[b]))
        else:
            idx16 = in_maps[-1]["idx"]
        in_maps.append({
            "x": np.ascontiguousarray(
                np.asarray(hidden_states[b, :, dh * DC:(dh + 1) * DC])
                .astype(ml_dtypes.bfloat16)),
            "p": np.ascontiguousarray(
                boundary_prob[b, :, 1], dtype=np.float32),
            "m": np.asarray(boundary_mask[b]).astype(np.uint8),
            "idx": idx16,
        })
    return in_maps


def _assemble(results):
    out = np.empty((B, L, D), np.float32)
    for c in range(NCORES):
        b, dh = c // 2, c % 2
        out[b, :, dh * DC:(dh + 1) * DC] = results[c]["out"].astype(np.float32)
    return out


def kernel(hidden_states, boundary_prob, boundary_mask, _run_kwargs=None):
    nc = _cache.get("nc")
    if nc is None:
        nc = _cache["nc"] = _build()
    in_maps = _in_maps(hidden_states, boundary_prob, boundary_mask)
    res = run_bass_kernel_spmd(nc, in_maps, core_ids=list(range(NCORES)),
                               **(_run_kwargs or {}))
    _cache["last_results"] = res
    return _assemble([res.results[c] for c in range(NCORES)])

